# revision 12
# baseline (speedup 1.0000x reference)
"""Trainium2 Bass kernel for additive-attention pooling.

  reference math (per sample b):
      scores = tanh(X[b] @ W) @ u          # (T,)
      att    = softmax(scores)             # (T,)
      out[b] = att @ X[b]                  # (D,)

  B, T, D, CTX = 32, 8192, 256, 128.

Strategy: data-parallel over batch, 4 samples per core on 8 cores.
X is uploaded in two layouts totalling 6 MB/sample (25% less HBM traffic
than one fp32 copy):
  XT8 - transposed fp8 e4m3 [p=d%128, h=d//128, t] -> feeds the
        d-contraction (scores) as the fp8 MOVING operand of a mixed-dtype
        matmul with the fp16 W halves stationary.  fp8 only perturbs the
        softmax weights; the pooled output error stays ~3e-3.  (W itself
        must stay fp16: its quantization noise is shared across t so it
        skips the 1/sqrt(T) softmax averaging that makes X-fp8 cheap.)
  XN  - natural fp16 [p=t%128, j=t//128, d] -> feeds the t-contraction
        (pooling) at full fp16 precision.  The 64 pooling matmuls per
        sample have M=1, so they are column-tiled 4-wide (tile_position
        (0,32g), g=j%4): four accumulation chains run concurrently in
        disjoint 32-column strips of the PE array, quartering pooling PE
        time.  The 4 partial rows (psum partitions 0/32/64/96) are
        stacked and summed on DVE before the final normalize.
Scores for chunk i are routed to PSUM partition row i via a masked copy of
u (only output-column i nonzero), accumulating all 16 chunk matmuls into one
[16, 512] PSUM tile.  exp and its row-sums are one fused ACT op.  softmax
normalization (divide by the scalar sum) is applied to the final 256-vector
instead of the 8192 attention weights.

The emission order software-pipelines the engines: the PE queue is strict
FIFO, so consumers of ACT/DVE results are emitted with a lag (u-matmuls lag
their scores pair by 2; transposes+att-copy lag their stage by 1; pooling
lags its stage by 2).  This keeps the PE streaming matmuls instead of
stalling on tanh/exp/att dependencies.
"""

import numpy as np

B, T, D, CTX = 32, 8192, 256, 128
N_CORES = 8
SPB = B // N_CORES  # samples per core

# Set True (e.g. from test.py) to capture an NTFF profile; LAST_RESULTS then
# holds the BassKernelResults with exec_time_ns.
TRACE = False
LAST_RESULTS = None

_prog_cache = {}


def _build_program(spb, t_len, nch, w16, umask, ident, ones_col, repeat=1,
                   parts="all", hw_loop=0, nsp=2):
    """Build + compile the SPMD Bass program.

    spb: samples per core; t_len: time length; nch: score chunks (chunk = t_len/nch).
    w16   [128, 2, CTX]   fp16  W in lhsT layout [p, h, c], h = d-half
    umask [nch, CTX, nch] fp16  umask[i, c, m] = u[c] * (m == i)
    ident [nch, nch]      fp16  identity for PE transposes
    ones_col [nch, 1]     fp32  ones, for the total-sum matmul
    """
    import concourse.bass as bass
    import concourse.tile as tile
    from concourse import bacc, mybir

    f8 = mybir.dt.float8e4
    f16 = mybir.dt.float16
    f32 = mybir.dt.float32
    AF = mybir.ActivationFunctionType

    ch = t_len // nch          # elements per score chunk (512)
    nt = t_len // 128          # number of 128-row t-tiles (64)
    jpc = ch // 128            # t-tiles per score chunk (4)
    nchg = nch // nsp          # chunks per group/stage (8)
    npr = nchg // 2            # score pairs per stage (4)
    jpg = nt // nsp            # t-tiles per stage (32)
    nstg = spb * nsp           # pipeline stages (8)
    npair = nstg * npr         # total score pairs (32)
    np_xt = 2                  # xt DMA pieces per sample (1 MB each)
    tpp = t_len // np_xt
    # sample 0 gets graduated pieces (256 KB first) to shorten pipeline fill
    pieces0 = [t_len // 8, t_len // 8 * 3, t_len // 2]
    def xt_pieces(s):
        return pieces0 if s == 0 else [tpp] * np_xt

    nc = bacc.Bacc("TRN2", target_bir_lowering=False, debug=False,
                   num_devices=N_CORES)

    XT8 = nc.dram_tensor("XT8", [spb, 128, 2, t_len], f8, kind="ExternalInput")
    XN = nc.dram_tensor("XN", [spb, 128, nt, D], f16, kind="ExternalInput")
    OUT = nc.dram_tensor("OUT", [spb, D], f32, kind="ExternalOutput")

    W_h = nc.inline_tensor(w16, name="Wc")
    UM_h = nc.inline_tensor(umask, name="UMc")
    ID_h = nc.inline_tensor(ident, name="IDc")
    ONE_h = nc.inline_tensor(ones_col, name="ONEc")

    with tile.TileContext(nc) as tc:
        with (
            tc.tile_pool(name="const", bufs=1) as cpool,
            tc.tile_pool(name="xt", bufs=3) as xt_pool,
            tc.tile_pool(name="xt0", bufs=2) as xt0_pool,
            tc.tile_pool(name="xn", bufs=2 * nsp) as xn_pool,
            tc.tile_pool(name="y", bufs=4) as y_pool,
            tc.tile_pool(name="sm", bufs=3) as sm_pool,
            tc.tile_pool(name="res", bufs=2) as res_pool,
            tc.tile_pool(name="py", bufs=2, space="PSUM") as py_pool,
            tc.tile_pool(name="ps", bufs=1, space="PSUM") as ps_pool,
            tc.tile_pool(name="pt", bufs=1, space="PSUM") as pt_pool,
            tc.tile_pool(name="po", bufs=1, space="PSUM") as po_pool,
        ):
            # ---- constants (resident) ----
            w_sb = cpool.tile([128, 2, CTX], f16, tag="w")
            nc.sync.dma_start(w_sb[:], W_h.ap())
            um_sb = cpool.tile([CTX, nch, nch], f16, tag="um")
            nc.sync.dma_start(um_sb[:], UM_h.ap().rearrange("i c m -> c i m"))
            id_sb = cpool.tile([nch, nch], f16, tag="id")
            nc.sync.dma_start(id_sb[:], ID_h.ap())
            one_sb = cpool.tile([nch, 1], f32, tag="one")
            nc.sync.dma_start(one_sb[:], ONE_h.ap())
            # dummy activation so the exp_and_others table load sits outside
            # the hardware loop body
            warm = cpool.tile([nch, 1], f32, tag="warm")
            nc.scalar.activation(warm[:], one_sb[:], AF.Tanh)

            def _one_repeat():
              out_sb = res_pool.tile([1, spb * D], f32, tag="out")
              xts, xns, ys, pss, exs, atts, ptots, pos = \
                  {}, {}, {}, {}, {}, {}, {}, {}

              def emit_dma(s):
                  lens = xt_pieces(s)
                  pool = xt0_pool if s == 0 else xt_pool
                  xts[s] = [pool.tile([128, 2, ln], f8, tag=f"xt{len(lens)}_{k}",
                                      name=f"xt{s}_{k}")
                            for k, ln in enumerate(lens)]
                  xns[s] = [xn_pool.tile([128, jpg, D], f16, tag="xn",
                                         name=f"xn{s}_{g}")
                            for g in range(nsp)]
                  if parts in ("all", "dma"):
                      o = 0
                      for k, ln in enumerate(lens):
                          nc.sync.dma_start(
                              xts[s][k][:],
                              XT8.ap()[s][:, :, o:o + ln])
                          o += ln
                      for g in range(nsp):
                          nc.sync.dma_start(
                              xns[s][g][:],
                              XN.ap()[s][:, g * jpg:(g + 1) * jpg, :])
                  else:
                      for k in range(len(lens)):
                          nc.vector.memset(xts[s][k][:, 0, 0:1], 0)
                      for g in range(nsp):
                          nc.vector.memset(xns[s][g][:, 0, 0:1], 0)

              def emit_sc(k):
                  # scores pair k: two 512-wide chunks through W (both halves),
                  # W loaded once per half (2 LDWEIGHTS per pair), then tanh.
                  # (fp16 W: fp8 W noise is shared across t and does NOT get
                  # the 1/sqrt(T) softmax averaging, costing ~1.6e-2 rel err.)
                  s, r = divmod(k, nsp * npr)
                  t0 = r * 2 * ch
                  kp, off = 0, 0
                  for ln in xt_pieces(s):
                      if t0 < off + ln:
                          break
                      kp += 1
                      off += ln
                  o0 = t0 - off
                  py2 = py_pool.tile([CTX, 2, ch], f32, tag="py")
                  for h in range(2):
                      for sub in range(2):
                          oo = o0 + sub * ch
                          nc.tensor.matmul(py2[:, sub, :], w_sb[:, h],
                                           xts[s][kp][:, h, oo:oo + ch],
                                           start=(h == 0), stop=(h == 1),
                                           skip_group_check=True)
                  y2 = y_pool.tile([CTX, 2, ch], f16, tag="y")
                  nc.scalar.activation(y2[:], py2[:], AF.Tanh)
                  ys[k] = y2

              def emit_u(k):
                  # u-contraction for pair k; chunk r of its stage lands in
                  # psum row r.  On the stage's last pair, fuse exp+row-sums.
                  i, pr = divmod(k, npr)
                  s, g = divmod(i, nsp)
                  if pr == 0:
                      pss[i] = ps_pool.tile([nchg, ch], f32, tag="ps", name=f"ps{i}")
                  ps = pss[i]
                  y2 = ys.pop(k)
                  for sub in range(2):
                      r = 2 * pr + sub
                      nc.tensor.matmul(ps[:],
                                       um_sb[:, g * nchg + r,
                                             g * nchg:(g + 1) * nchg],
                                       start=(r == 0), stop=(r == nchg - 1),
                                       rhs=y2[:, sub, :],
                                       skip_group_check=True)
                  if pr == npr - 1:
                      ex = sm_pool.tile([nchg, ch], f16, tag="ex")
                      sums = sm_pool.tile([nchg, 1], f32, tag="sums")
                      nc.scalar.activation(ex[:], ps[:], AF.Exp,
                                           accum_out=sums[:])
                      exs[i] = (ex, sums)

              def emit_mid(i):
                  # transpose exp chunks into att columns; accumulate total.
                  s, g = divmod(i, nsp)
                  ex, sums = exs.pop(i)
                  pt = pt_pool.tile([128, jpc, nchg], f16, tag="pt")
                  for q in range(jpc):
                      nc.tensor.transpose(pt[:, q, :],
                                          ex[:, q * 128:(q + 1) * 128],
                                          id_sb[0:nchg, 0:nchg])
                  att = sm_pool.tile([128, jpc * nchg], f16, tag="att")
                  nc.vector.tensor_copy(att[:], pt[:])
                  atts[i] = att
                  if g == 0:
                      ptots[s] = pt_pool.tile([1, 1], f32, tag="ptot", name=f"ptot{s}")
                  nc.tensor.matmul(ptots[s][:], one_sb[0:nchg, :], sums[:],
                                   start=(g == 0), stop=(g == nsp - 1),
                                   skip_group_check=True)

              def emit_b(i):
                  # pooling share of stage i, column-tiled 4-wide: group
                  # g=j%4 accumulates in PE column strip 32g into psum row
                  # 32g.  On the sample's last stage, stack the 4 partial
                  # rows, sum, and normalize into the output row.
                  s, g = divmod(i, nsp)
                  if g == 0:
                      pos[s] = po_pool.tile([128, D], f32, tag="po", name=f"po{s}")
                  po = pos[s]
                  att = atts.pop(i)
                  for jj in range(jpg):
                      j = g * jpg + jj
                      b_, q_ = divmod(jj, jpc)
                      col = q_ * nchg + b_
                      grp = j % 4
                      nc.tensor.matmul(po[32 * grp:32 * grp + 1, :],
                                       att[:, col:col + 1],
                                       xns[s][g][:, jj, :],
                                       start=(j < 4), stop=(j >= nt - 4),
                                       tile_position=(0, 32 * grp),
                                       skip_group_check=True)
                  if g == nsp - 1:
                      inv = sm_pool.tile([1, 1], f32, tag="inv")
                      nc.vector.reciprocal(inv[:], ptots.pop(s)[:])
                      m4 = sm_pool.tile([1, 4, D], f32, tag="m4")
                      for grp in range(4):
                          nc.vector.tensor_copy(m4[:, grp, :],
                                                po[32 * grp:32 * grp + 1, :])
                      s01 = sm_pool.tile([1, D], f32, tag="s01")
                      nc.vector.scalar_tensor_tensor(
                          s01[:], m4[:, 0, :], 0.0, m4[:, 1, :],
                          mybir.AluOpType.add, mybir.AluOpType.add)
                      s23 = sm_pool.tile([1, D], f32, tag="s23")
                      nc.vector.scalar_tensor_tensor(
                          s23[:], m4[:, 2, :], 0.0, m4[:, 3, :],
                          mybir.AluOpType.add, mybir.AluOpType.add)
                      stot = sm_pool.tile([1, D], f32, tag="stot")
                      nc.vector.scalar_tensor_tensor(
                          stot[:], s01[:], 0.0, s23[:],
                          mybir.AluOpType.add, mybir.AluOpType.add)
                      nc.vector.tensor_scalar(out_sb[0:1, s * D:(s + 1) * D],
                                              stot[:], inv[:], None,
                                              mybir.AluOpType.mult)

              if parts == "dma":
                  for s in range(spb):
                      emit_dma(s)
                  nc.vector.memset(out_sb[:, 0:1], 0)
              else:
                  for k in range(npair + 2):
                      if k < npair:
                          if k % (nsp * npr) == 0:
                              emit_dma(k // (nsp * npr))
                          emit_sc(k)
                      ku = k - 2
                      if ku >= 0 and ku < npair:
                          emit_u(ku)
                          if ku % npr == npr - 1:
                              i = ku // npr
                              if i - 2 >= 0:
                                  emit_b(i - 2)
                              if i - 1 >= 0:
                                  emit_mid(i - 1)
                  emit_mid(nstg - 1)
                  emit_b(nstg - 2)
                  emit_b(nstg - 1)

              nc.sync.dma_start(OUT.ap().rearrange("s d -> () (s d)"), out_sb[:])

            if hw_loop:
                with tc.For_i(0, hw_loop, 1,
                              hint_engines=(mybir.EngineType.PE,
                                            mybir.EngineType.Activation,
                                            mybir.EngineType.SP,
                                            mybir.EngineType.DVE)):
                    for _r in range(repeat):
                        _one_repeat()
            else:
                for _r in range(repeat):
                    _one_repeat()

    nc.compile()
    return nc


def _prep_consts(W, u, nch):
    # lhsT layout [p, h, c]: w16[p, h, c] = W[h*128+p, c]
    w16 = np.ascontiguousarray(
        W.reshape(2, 128, CTX).transpose(1, 0, 2)).astype(np.float16)
    u16 = u.astype(np.float16).reshape(CTX)
    umask = np.zeros((nch, CTX, nch), dtype=np.float16)
    for i in range(nch):
        umask[i, :, i] = u16
    ident = np.eye(nch, dtype=np.float16)
    ones_col = np.ones((nch, 1), dtype=np.float32)
    return w16, umask, ident, ones_col


def _pack_inputs(Xs):
    """Xs: [nb, T, D] fp32 -> (XT8 [nb,128,2,T] fp8, XN [nb,128,T//128,D] fp16)."""
    import ml_dtypes
    nb, t_len, d = Xs.shape
    # XT8[s, p, h, t] = X[s, t, h*128+p]
    xt = np.ascontiguousarray(
        Xs.transpose(0, 2, 1).reshape(nb, 2, 128, t_len).transpose(0, 2, 1, 3)
    ).astype(ml_dtypes.float8_e4m3)
    # XN[s, p, j, d] = X[s, j*128+p, d]
    xn = np.ascontiguousarray(
        Xs.astype(np.float16).reshape(nb, t_len // 128, 128, d).transpose(0, 2, 1, 3))
    return xt, xn


def kernel(X, W, u):
    global LAST_RESULTS
    from concourse.bass_utils import run_bass_kernel_spmd

    X = np.asarray(X)
    W = np.asarray(W)
    u = np.asarray(u)
    assert X.shape == (B, T, D) and W.shape == (D, CTX) and u.shape == (CTX, 1), (
        X.shape, W.shape, u.shape)

    nch = 16
    key = (SPB, T, nch, W.tobytes(), u.tobytes())
    if key not in _prog_cache:
        _prog_cache.clear()
        _prog_cache[key] = _build_program(
            SPB, T, nch, *_prep_consts(W, u, nch))
    nc = _prog_cache[key]

    in_maps = []
    for c in range(N_CORES):
        xt, xn = _pack_inputs(X[c * SPB:(c + 1) * SPB])
        in_maps.append({"XT8": xt, "XN": xn})

    try:
        res = run_bass_kernel_spmd(nc, in_maps, core_ids=list(range(N_CORES)),
                                   trace=TRACE)
    except (ImportError, ModuleNotFoundError):
        # NTFF profiling hook unavailable in this axon build; run untraced.
        res = run_bass_kernel_spmd(nc, in_maps, core_ids=list(range(N_CORES)),
                                   trace=False)
    LAST_RESULTS = res
    return np.concatenate([r["OUT"] for r in res.results], axis=0)



# revision 25
# speedup vs baseline: 1.0455x; 1.0455x over previous
"""Trainium2 Bass kernel for additive-attention pooling.

  reference math (per sample b):
      scores = tanh(X[b] @ W) @ u          # (T,)
      att    = softmax(scores)             # (T,)
      out[b] = att @ X[b]                  # (D,)

  B, T, D, CTX = 32, 8192, 256, 128.

Strategy: data-parallel over batch, 4 samples per core on 8 cores.
X is uploaded in two layouts totalling 6 MB/sample (25% less HBM traffic
than one fp32 copy):
  XT8 - transposed fp8 e4m3 [p=d%128, h=d//128, t] -> feeds the
        d-contraction (scores) as the fp8 MOVING operand of a mixed-dtype
        matmul with the fp16 W halves stationary.  fp8 only perturbs the
        softmax weights; the pooled output error stays ~3e-3.  (W itself
        must stay fp16: its quantization noise is shared across t so it
        skips the 1/sqrt(T) softmax averaging that makes X-fp8 cheap.)
  XN  - natural fp16 [p=t%128, j=t//128, d] -> feeds the t-contraction
        (pooling) at full fp16 precision.  The 64 pooling matmuls per
        sample have M=1, so they are column-tiled 4-wide (tile_position
        (0,32g), g=j%4): four accumulation chains run concurrently in
        disjoint 32-column strips of the PE array, quartering pooling PE
        time.  The 4 partial rows (psum partitions 0/32/64/96) are
        stacked and summed on DVE before the final normalize.
Scores for chunk i are routed to PSUM partition row i via a masked copy of
u (only output-column i nonzero), accumulating all 16 chunk matmuls into one
[16, 512] PSUM tile.  exp and its row-sums are one fused ACT op.  softmax
normalization (divide by the scalar sum) is applied to the final 256-vector
instead of the 8192 attention weights.

The emission order software-pipelines the engines: the PE queue is strict
FIFO, so consumers of ACT/DVE results are emitted with a lag (u-matmuls lag
their scores pair by 2; transposes+att-copy lag their stage by 1; pooling
lags its stage by 2).  This keeps the PE streaming matmuls instead of
stalling on tanh/exp/att dependencies.
"""

import numpy as np

B, T, D, CTX = 32, 8192, 256, 128
N_CORES = 8
SPB = B // N_CORES  # samples per core

# Set True (e.g. from test.py) to capture an NTFF profile; LAST_RESULTS then
# holds the BassKernelResults with exec_time_ns.
TRACE = False
LAST_RESULTS = None

_prog_cache = {}


def _build_program(spb, t_len, nch, w16, umask, ident, ones_col, repeat=1,
                   parts="all", hw_loop=0, nsp=2):
    """Build + compile the SPMD Bass program.

    spb: samples per core; t_len: time length; nch: score chunks (chunk = t_len/nch).
    w16   [128, 2, CTX]  fp16  W in lhsT layout [p, h, c], h = d-half
    umask [CTX, 2, 32]   fp16  umask[c, r, m] = u[c] * (m == r): round-r
                               stationary for the 4-wide col-tiled u-matmuls
                               (all 32 strip rows written; zeros elsewhere)
    ident [128, 128]     fp16  identity for PE transposes
    ones_col [128, 1]    fp32  1 at the 8 valid score rows {32g+r}, else 0
    """
    import concourse.bass as bass
    import concourse.tile as tile
    from concourse import bacc, mybir

    f8 = mybir.dt.float8e4
    f16 = mybir.dt.float16
    f32 = mybir.dt.float32
    AF = mybir.ActivationFunctionType

    ch = t_len // nch          # elements per score chunk (512)
    nt = t_len // 128          # number of 128-row t-tiles (64)
    jpc = ch // 128            # t-tiles per score chunk (4)
    nchg = nch // nsp          # chunks per group/stage (8)
    npr = nchg // 2            # score pairs per stage (4)
    jpg = nt // nsp            # t-tiles per stage (32)
    nstg = spb * nsp           # pipeline stages (8)
    npair = nstg * npr         # total score pairs (32)
    np_xt = 1                  # xt DMA pieces per sample (2 MB, 16 KB runs)
    tpp = t_len // np_xt
    # sample 0 gets graduated pieces (256 KB first) to shorten pipeline fill
    pieces0 = [t_len // 8, t_len // 8 * 3, t_len // 2]
    def xt_pieces(s):
        return pieces0 if s == 0 else [tpp] * np_xt

    nc = bacc.Bacc("TRN2", target_bir_lowering=False, debug=False,
                   num_devices=N_CORES)

    XT8 = nc.dram_tensor("XT8", [spb, 128, t_len, 2], f8, kind="ExternalInput")
    XN = nc.dram_tensor("XN", [spb, 128, nt, D], f16, kind="ExternalInput")
    OUT = nc.dram_tensor("OUT", [spb, D], f32, kind="ExternalOutput")

    W_h = nc.inline_tensor(w16, name="Wc")
    UM_h = nc.inline_tensor(umask, name="UMc")
    ID_h = nc.inline_tensor(ident, name="IDc")
    ONE_h = nc.inline_tensor(ones_col, name="ONEc")

    with tile.TileContext(nc) as tc:
        with (
            tc.tile_pool(name="const", bufs=1) as cpool,
            tc.tile_pool(name="xt", bufs=3) as xt_pool,
            tc.tile_pool(name="xt0", bufs=2) as xt0_pool,
            tc.tile_pool(name="xn", bufs=2 * nsp) as xn_pool,
            tc.tile_pool(name="y", bufs=4) as y_pool,
            tc.tile_pool(name="sm", bufs=3) as sm_pool,
            tc.tile_pool(name="res", bufs=2) as res_pool,
            tc.tile_pool(name="py", bufs=2, space="PSUM") as py_pool,
            tc.tile_pool(name="ps", bufs=1, space="PSUM") as ps_pool,
            tc.tile_pool(name="pt", bufs=1, space="PSUM") as pt_pool,
            tc.tile_pool(name="po", bufs=1, space="PSUM") as po_pool,
        ):
            # ---- constants (resident) ----
            w_sb = cpool.tile([128, 2, CTX], f16, tag="w")
            nc.sync.dma_start(w_sb[:], W_h.ap())
            um_sb = cpool.tile([CTX, 2, 32], f16, tag="um")
            nc.sync.dma_start(um_sb[:], UM_h.ap())
            id_sb = cpool.tile([128, 128], f16, tag="id")
            nc.sync.dma_start(id_sb[:], ID_h.ap())
            one_sb = cpool.tile([128, 1], f32, tag="one")
            nc.sync.dma_start(one_sb[:], ONE_h.ap())
            # dummy activation so the exp_and_others table load sits outside
            # the hardware loop body
            warm = cpool.tile([128, 1], f32, tag="warm")
            nc.scalar.activation(warm[:], one_sb[:], AF.Tanh)

            def _one_repeat():
              out_sb = res_pool.tile([1, spb * D], f32, tag="out")
              xts, xns, ys, pss, exs, atts, ptots, pos = \
                  {}, {}, {}, {}, {}, {}, {}, {}

              def emit_dma(s):
                  lens = xt_pieces(s)
                  pool = xt0_pool if s == 0 else xt_pool
                  xts[s] = [pool.tile([128, ln, 2], f8, tag=f"xt{len(lens)}_{k}",
                                      name=f"xt{s}_{k}")
                            for k, ln in enumerate(lens)]
                  xns[s] = [xn_pool.tile([128, jpg, D], f16, tag="xn",
                                         name=f"xn{s}_{g}")
                            for g in range(nsp)]
                  if parts in ("all", "dma"):
                      o = 0
                      for k, ln in enumerate(lens):
                          nc.sync.dma_start(
                              xts[s][k][:],
                              XT8.ap()[s][:, o:o + ln, :])
                          o += ln
                      for g in range(nsp):
                          nc.sync.dma_start(
                              xns[s][g][:],
                              XN.ap()[s][:, g * jpg:(g + 1) * jpg, :])
                  else:
                      for k in range(len(lens)):
                          nc.vector.memset(xts[s][k][:, 0, 0:1], 0)
                      for g in range(nsp):
                          nc.vector.memset(xns[s][g][:, 0, 0:1], 0)

              def emit_sc(k):
                  # scores pair k: two 512-wide chunks through W (both halves),
                  # W loaded once per half (2 LDWEIGHTS per pair), then tanh.
                  # (fp16 W: fp8 W noise is shared across t and does NOT get
                  # the 1/sqrt(T) softmax averaging, costing ~1.6e-2 rel err.)
                  s, r = divmod(k, nsp * npr)
                  t0 = r * 2 * ch
                  kp, off = 0, 0
                  for ln in xt_pieces(s):
                      if t0 < off + ln:
                          break
                      kp += 1
                      off += ln
                  o0 = t0 - off
                  py2 = py_pool.tile([CTX, 2, ch], f32, tag="py")
                  for h in range(2):
                      for sub in range(2):
                          oo = o0 + sub * ch
                          nc.tensor.matmul(py2[:, sub, :], w_sb[:, h],
                                           xts[s][kp][:, oo:oo + ch, h],
                                           start=(h == 0), stop=(h == 1),
                                           skip_group_check=True)
                  y2 = y_pool.tile([CTX, 2, ch], f16, tag="y")
                  nc.scalar.activation(y2[:], py2[:], AF.Tanh)
                  ys[k] = y2

              def emit_u(k):
                  # u-contraction for pair k, col-tiled 4-wide: chunk c of
                  # the stage lands in psum row 32*(c%4) + c//4 via strip
                  # (c%4) and a masked-u stationary with column c//4 = u.
                  # On the stage's last pair, fuse exp+row-sums (all 128
                  # rows are strip-written, so no stale-psum garbage).
                  i, pr = divmod(k, npr)
                  s, g = divmod(i, nsp)
                  if pr == 0:
                      pss[i] = ps_pool.tile([128, ch], f32, tag="ps", name=f"ps{i}")
                  ps = pss[i]
                  y2 = ys.pop(k)
                  for sub in range(2):
                      c = 2 * pr + sub
                      grp, rr = c % 4, c // 4
                      nc.tensor.matmul(ps[32 * grp:32 * grp + 32, :],
                                       um_sb[:, rr, :],
                                       start=(rr == 0), stop=(rr == 1),
                                       rhs=y2[:, sub, :],
                                       tile_position=(0, 32 * grp),
                                       skip_group_check=True)
                  if pr == npr - 1:
                      ex = sm_pool.tile([128, ch], f16, tag="ex")
                      sums = sm_pool.tile([128, 1], f32, tag="sums")
                      nc.scalar.activation(ex[:], ps[:], AF.Exp,
                                           accum_out=sums[:])
                      exs[i] = (ex, sums)

              def emit_mid(i):
                  # transpose exp chunks into att columns (128-wide; only
                  # the 8 strip rows {32g+r} carry data) and gather the
                  # valid columns with a strided 4D copy; accumulate total
                  # via the masked-ones column (zeros kill the junk rows).
                  s, g = divmod(i, nsp)
                  ex, sums = exs.pop(i)
                  pt = pt_pool.tile([128, jpc, 128], f16, tag="pt")
                  for q in range(jpc):
                      nc.tensor.transpose(pt[:, q, :],
                                          ex[:, q * 128:(q + 1) * 128],
                                          id_sb[:])
                  att = sm_pool.tile([128, jpc, 4, 2], f16, tag="att")
                  nc.vector.tensor_copy(
                      att[:],
                      pt[:].rearrange("p q (g rr) -> p q g rr", g=4)[:, :, :, 0:2])
                  atts[i] = att
                  if g == 0:
                      ptots[s] = pt_pool.tile([1, 1], f32, tag="ptot", name=f"ptot{s}")
                  nc.tensor.matmul(ptots[s][:], one_sb[:], sums[:],
                                   start=(g == 0), stop=(g == nsp - 1),
                                   skip_group_check=True)

              def emit_b(i):
                  # pooling share of stage i, column-tiled 4-wide: group
                  # g=j%4 accumulates in PE column strip 32g into psum row
                  # 32g.  On the sample's last stage, stack the 4 partial
                  # rows, sum, and normalize into the output row.
                  s, g = divmod(i, nsp)
                  if g == 0:
                      pos[s] = po_pool.tile([128, D], f32, tag="po", name=f"po{s}")
                  po = pos[s]
                  att = atts.pop(i)
                  for jj in range(jpg):
                      j = g * jpg + jj
                      b_, q_ = divmod(jj, jpc)
                      grp = j % 4
                      nc.tensor.matmul(po[32 * grp:32 * grp + 1, :],
                                       att[:, q_, b_ % 4, b_ // 4:b_ // 4 + 1],
                                       xns[s][g][:, jj, :],
                                       start=(j < 4), stop=(j >= nt - 4),
                                       tile_position=(0, 32 * grp),
                                       skip_group_check=True)
                  if g == nsp - 1:
                      inv = sm_pool.tile([1, 1], f32, tag="inv")
                      nc.vector.reciprocal(inv[:], ptots.pop(s)[:])
                      m4 = sm_pool.tile([1, 4, D], f32, tag="m4")
                      for grp in range(4):
                          nc.vector.tensor_copy(m4[:, grp, :],
                                                po[32 * grp:32 * grp + 1, :])
                      s01 = sm_pool.tile([1, D], f32, tag="s01")
                      nc.vector.scalar_tensor_tensor(
                          s01[:], m4[:, 0, :], 0.0, m4[:, 1, :],
                          mybir.AluOpType.add, mybir.AluOpType.add)
                      s23 = sm_pool.tile([1, D], f32, tag="s23")
                      nc.vector.scalar_tensor_tensor(
                          s23[:], m4[:, 2, :], 0.0, m4[:, 3, :],
                          mybir.AluOpType.add, mybir.AluOpType.add)
                      stot = sm_pool.tile([1, D], f32, tag="stot")
                      nc.vector.scalar_tensor_tensor(
                          stot[:], s01[:], 0.0, s23[:],
                          mybir.AluOpType.add, mybir.AluOpType.add)
                      nc.vector.tensor_scalar(out_sb[0:1, s * D:(s + 1) * D],
                                              stot[:], inv[:], None,
                                              mybir.AluOpType.mult)

              if parts == "dma":
                  for s in range(spb):
                      emit_dma(s)
                  nc.vector.memset(out_sb[:, 0:1], 0)
              else:
                  for k in range(npair + 2):
                      if k < npair:
                          if k % (nsp * npr) == 0:
                              emit_dma(k // (nsp * npr))
                          emit_sc(k)
                      ku = k - 2
                      if ku >= 0 and ku < npair:
                          emit_u(ku)
                          if ku % npr == npr - 1:
                              i = ku // npr
                              if i - 2 >= 0:
                                  emit_b(i - 2)
                              if i - 1 >= 0:
                                  emit_mid(i - 1)
                  emit_mid(nstg - 1)
                  emit_b(nstg - 2)
                  emit_b(nstg - 1)

              nc.sync.dma_start(OUT.ap().rearrange("s d -> () (s d)"), out_sb[:])

            if hw_loop:
                with tc.For_i(0, hw_loop, 1,
                              hint_engines=(mybir.EngineType.PE,
                                            mybir.EngineType.Activation,
                                            mybir.EngineType.SP,
                                            mybir.EngineType.DVE)):
                    for _r in range(repeat):
                        _one_repeat()
            else:
                for _r in range(repeat):
                    _one_repeat()

    nc.compile()
    return nc


def _prep_consts(W, u, nch):
    # lhsT layout [p, h, c]: w16[p, h, c] = W[h*128+p, c]
    w16 = np.ascontiguousarray(
        W.reshape(2, 128, CTX).transpose(1, 0, 2)).astype(np.float16)
    u16 = u.astype(np.float16).reshape(CTX)
    umask = np.zeros((CTX, 2, 32), dtype=np.float16)
    for r in range(2):
        umask[:, r, r] = u16
    ident = np.eye(128, dtype=np.float16)
    ones_col = np.zeros((128, 1), dtype=np.float32)
    for g in range(4):
        for r in range(2):
            ones_col[32 * g + r, 0] = 1.0
    return w16, umask, ident, ones_col


def _pack_inputs(Xs):
    """Xs: [nb, T, D] fp32 -> (XT8 [nb,128,T,2] fp8, XN [nb,128,T//128,D] fp16)."""
    import ml_dtypes
    nb, t_len, d = Xs.shape
    # XT8[s, p, t, h] = X[s, t, h*128+p]  (h pairs adjacent: 16 KB
    # contiguous per partition per sample -> efficient DMA descriptors)
    xt = np.ascontiguousarray(
        Xs.transpose(0, 2, 1).reshape(nb, 2, 128, t_len).transpose(0, 2, 3, 1)
    ).astype(ml_dtypes.float8_e4m3)
    # XN[s, p, j, d] = X[s, j*128+p, d]
    xn = np.ascontiguousarray(
        Xs.astype(np.float16).reshape(nb, t_len // 128, 128, d).transpose(0, 2, 1, 3))
    return xt, xn


def kernel(X, W, u):
    global LAST_RESULTS
    from concourse.bass_utils import run_bass_kernel_spmd

    X = np.asarray(X)
    W = np.asarray(W)
    u = np.asarray(u)
    assert X.shape == (B, T, D) and W.shape == (D, CTX) and u.shape == (CTX, 1), (
        X.shape, W.shape, u.shape)

    nch = 16
    key = (SPB, T, nch, W.tobytes(), u.tobytes())
    if key not in _prog_cache:
        _prog_cache.clear()
        _prog_cache[key] = _build_program(
            SPB, T, nch, *_prep_consts(W, u, nch))
    nc = _prog_cache[key]

    in_maps = []
    for c in range(N_CORES):
        xt, xn = _pack_inputs(X[c * SPB:(c + 1) * SPB])
        in_maps.append({"XT8": xt, "XN": xn})

    try:
        res = run_bass_kernel_spmd(nc, in_maps, core_ids=list(range(N_CORES)),
                                   trace=TRACE)
    except (ImportError, ModuleNotFoundError):
        # NTFF profiling hook unavailable in this axon build; run untraced.
        res = run_bass_kernel_spmd(nc, in_maps, core_ids=list(range(N_CORES)),
                                   trace=False)
    LAST_RESULTS = res
    return np.concatenate([r["OUT"] for r in res.results], axis=0)

